# revision 1
# baseline (speedup 1.0000x reference)
"""Trainium2 Bass kernel for a dense transformer block (B=2, T=2048, D=768, H=12).

Sharding: 8 cores, each owns 512 contiguous tokens of one batch element
(4 cores per batch).  Each core receives its batch's full token stream
(rotated so its own 512 query rows come first), computes K/V for all 2048
keys of that batch locally (no cross-core communication), and attention +
FFN for its own 512 rows.  Host gathers the 8 row-slices.

LayerNorm affine params and the attention scale are folded into the weight
matrices on the host, so the device only computes pure (x-mean)*rstd
normalizations.
"""

import os
import numpy as np

import concourse.bass as bass
import concourse.tile as tile
from concourse import bacc, mybir
from concourse.bass_utils import run_bass_kernel_spmd
from concourse.masks import make_identity

F32 = mybir.dt.float32
BF = mybir.dt.bfloat16

D = 768
H = 12
HS = 64
B = 2
T = 2048
P = 128
NCORES = 8
CPB = NCORES // B          # cores per batch
ROWS = T // CPB            # 512 query rows per core
ST = T // P                # 16 key tiles
QT_N = ROWS // P           # 4 query tiles
DT_N = D // P              # 6 feature tiles
FF = 4 * D                 # 3072
FFT = FF // P              # 24 ff tiles
HP = H // 2                # 6 head pairs
EPS = 1e-5

# matmul input dtype: float32 (exact, 4 cyc/row) or float32r (~1 cyc/row)
MM_DT = {
    "f32": mybir.dt.float32,
    "f32r": mybir.dt.float32r,
}[os.environ.get("KERNEL_MM_DT", "f32r")]


MDT = MM_DT  # dtype of tiles that feed matmuls (producers round to f32r)


def _mm(nc, out, lhsT, rhs, **kw):
    nc.tensor.matmul(out, lhsT, rhs, **kw)


def build_nc(reps=None, skip_bias=()):
    nc = bacc.Bacc("TRN2", target_bir_lowering=False, debug=False, num_devices=NCORES)

    xb = nc.declare_dram_parameter("xb", [T, D], F32, isOutput=False)
    xsl = nc.declare_dram_parameter("xsl", [ROWS, D], F32, isOutput=False)
    maskf = nc.declare_dram_parameter("maskf", [T, 1], F32, isOutput=False)
    wq = nc.declare_dram_parameter("wq", [D, D], BF, isOutput=False)
    wk = nc.declare_dram_parameter("wk", [D, D], BF, isOutput=False)
    wv = nc.declare_dram_parameter("wv", [D, D], BF, isOutput=False)
    wo = nc.declare_dram_parameter("wo", [D, D], F32, isOutput=False)
    w1 = nc.declare_dram_parameter("w1", [D, FF], F32, isOutput=False)
    w2 = nc.declare_dram_parameter("w2", [FF, D], F32, isOutput=False)
    bq = nc.declare_dram_parameter("bq", [1, D], F32, isOutput=False)
    bk = nc.declare_dram_parameter("bk", [1, D], F32, isOutput=False)
    bv = nc.declare_dram_parameter("bv", [1, D], F32, isOutput=False)
    bo = nc.declare_dram_parameter("bo", [1, D], F32, isOutput=False)
    b1 = nc.declare_dram_parameter("b1", [FF], F32, isOutput=False)
    b2 = nc.declare_dram_parameter("b2", [1, D], F32, isOutput=False)
    y = nc.declare_dram_parameter("y", [ROWS, D], F32, isOutput=True)

    if reps is None:
        reps = int(os.environ.get("KERNEL_REPS", "1"))
    with tile.TileContext(nc) as tc, \
            nc.allow_low_precision(reason="f32r-rounded matmul operands"):
        for r in range(reps):
            if r:
                tc.strict_bb_all_engine_barrier()
            _emit(nc, tc, xb, xsl, maskf, wq, wk, wv, wo, w1, w2,
                  bq, bk, bv, bo, b1, b2, y, skip_bias=frozenset(skip_bias))
    nc.compile()
    return nc


def _layernorm_tiles(nc, pool, tiles, n, eps_t, outs=None, act_frac=0.0):
    """(x-mean)*rstd over `tiles[:n]` ([P, D] token-major), in place unless
    `outs` supplies destination tiles.  A leading fraction of the tiles is
    handled on the ACT engine (Square/Identity passes with accum_out) so DVE
    and ACT split the work."""
    n_act = int(n * act_frac)
    for i in range(n):
        xt = tiles[i]
        out_t = xt if outs is None else outs[i]
        std = pool.tile([P, 1], F32, name="lnstd", tag="lnstd", bufs=4)
        if i < n_act:
            scr = pool.tile([P, D], F32, name="lnscr", tag="lnscr", bufs=1)
            sums = pool.tile([P, 2], F32, name="lnsums", tag="lnsums", bufs=4)
            nc.scalar.activation(out=scr, in_=xt,
                                 func=mybir.ActivationFunctionType.Square,
                                 accum_out=sums[:, 1:2])
            nc.scalar.activation(out=scr, in_=xt,
                                 func=mybir.ActivationFunctionType.Identity,
                                 accum_out=sums[:, 0:1])
            mv = pool.tile([P, 2], F32, name="lnmv", tag="lnmv", bufs=4)
            # mean = sum/D ; var = sumsq/D - mean^2
            nc.vector.tensor_scalar_mul(out=mv, in0=sums, scalar1=1.0 / D)
            m2 = pool.tile([P, 1], F32, name="lnm2", tag="lnm2", bufs=4)
            nc.vector.tensor_mul(out=m2, in0=mv[:, 0:1], in1=mv[:, 0:1])
            nc.vector.tensor_sub(out=mv[:, 1:2], in0=mv[:, 1:2], in1=m2)
            nc.scalar.activation(out=std, in_=mv[:, 1:2],
                                 func=mybir.ActivationFunctionType.Sqrt,
                                 bias=eps_t)
            nc.vector.reciprocal(out=std, in_=std)
            nmr = pool.tile([P, 1], F32, name="lnnmr", tag="lnnmr", bufs=4)
            nc.vector.scalar_tensor_tensor(out=nmr, in0=mv[:, 0:1],
                                           scalar=-1.0, in1=std,
                                           op0=mybir.AluOpType.mult,
                                           op1=mybir.AluOpType.mult)
            nc.scalar.activation(out=out_t, in_=xt,
                                 func=mybir.ActivationFunctionType.Identity,
                                 bias=nmr, scale=std)
        else:
            stats = pool.tile([P, 3, 6], F32, name="lnstats", tag="lnstats",
                              bufs=4)
            for g in range(3):
                nc.vector.bn_stats(out=stats[:, g, :],
                                   in_=xt[:, g * 256:(g + 1) * 256])
            mv = pool.tile([P, 2], F32, name="lnmv", tag="lnmv", bufs=4)
            nc.vector.bn_aggr(out=mv, in_=stats)
            nc.scalar.activation(out=std, in_=mv[:, 1:2],
                                 func=mybir.ActivationFunctionType.Sqrt,
                                 bias=eps_t)
            nc.vector.reciprocal(out=std, in_=std)
            nc.vector.tensor_scalar(out=out_t, in0=xt,
                                    scalar1=mv[:, 0:1], scalar2=std,
                                    op0=mybir.AluOpType.subtract,
                                    op1=mybir.AluOpType.mult)


def _emit(nc, tc, xb, xsl, maskf, wq, wk, wv, wo, w1, w2,
          bq, bk, bv, bo, b1, b2, y, skip_bias=frozenset()):
    # Pools are released in strict LIFO order per (space, side) stack; the
    # open/close sequence below alternates sides so that overlapping
    # non-nested lifetimes land on different stacks.
    def open_pool(name, bufs, space="SBUF", side=None):
        cm = tc.tile_pool(name=name, bufs=bufs, space=space, side=side)
        return cm, cm.__enter__()

    # ---------------- constants ----------------
    cm_const, consts = open_pool("consts", 1)
    identity = consts.tile([P, P], F32, name="identity", tag="identity")
    make_identity(nc, identity)
    identity_r = consts.tile([P, P], MDT, name="identity_r", tag="identity_r")
    nc.vector.tensor_copy(out=identity_r, in_=identity)
    ones_st = consts.tile([1, 512 + 3 * P], F32, name="ones_st", tag="ones_st")
    nc.vector.memset(ones_st[:, 0:512 + P], 1.0)
    nc.vector.memset(ones_st[:, 512 + P:], 0.0)
    nc.vector.memset(ones_st[:, 512 + P:512 + P + 64], 1.0)
    nc.vector.memset(ones_st[:, 512 + 2 * P + 64:], 1.0)
    ones_row = consts.tile([1, 512], MDT, name="ones_row", tag="ones_row")
    nc.vector.tensor_copy(out=ones_row, in_=ones_st[:, 0:512])
    onesfull = consts.tile([1, P], MDT, name="onesfull", tag="onesfull")
    nc.vector.tensor_copy(out=onesfull, in_=ones_st[:, 512:512 + P])
    onesA = consts.tile([1, P], MDT, name="onesA", tag="onesA")
    nc.vector.tensor_copy(out=onesA, in_=ones_st[:, 512 + P:512 + 2 * P])
    onesB = consts.tile([1, P], MDT, name="onesB", tag="onesB")
    nc.vector.tensor_copy(out=onesB, in_=ones_st[:, 512 + 2 * P:512 + 3 * P])
    eps_t = consts.tile([P, 1], F32, name="eps_t", tag="eps_t")
    nc.vector.memset(eps_t, EPS)
    ones_col = consts.tile([P, H], F32, name="ones_col", tag="ones_col")
    nc.vector.memset(ones_col, 1.0)

    cm_small, small = open_pool("small", 4)

    # ---------------- stage A: load x batch + LN1 (in place) ----------------
    # x tiles are loaded first (alternating HWDGE queues) so LN + transposes
    # start as early as possible; small const loads follow.
    cm_xn, xnp = open_pool("xn", ST)
    cm_xraw, xrawp = open_pool("xraw", 6)
    xn = [xnp.tile([P, D], MDT, name="xn", tag="xn", bufs=ST)
          for _ in range(ST)]
    xraw = []
    for i in range(ST):
        t = xrawp.tile([P, D], F32, name="xraw", tag="xraw", bufs=6)
        eng = nc.sync if i % 2 == 0 else nc.scalar
        eng.dma_start(out=t, in_=xb[i * P:(i + 1) * P, :])
        xraw.append(t)

    mask_all = consts.tile([P, ST], F32, name="mask_all", tag="mask_all")
    nc.sync.dma_start(out=mask_all, in_=maskf[:, :].rearrange("(n p) o -> p (n o)", p=P))
    bq_t = consts.tile([1, D], MDT, name="bq_t", tag="bq_t")
    nc.sync.dma_start(out=bq_t, in_=bq[:, :].bitcast(MDT))
    bk_t = consts.tile([1, D], MDT, name="bk_t", tag="bk_t")
    nc.sync.dma_start(out=bk_t, in_=bk[:, :].bitcast(MDT))
    bv_t = consts.tile([1, D], MDT, name="bv_t", tag="bv_t")
    nc.sync.dma_start(out=bv_t, in_=bv[:, :].bitcast(MDT))
    bo_t = consts.tile([1, D], MDT, name="bo_t", tag="bo_t")
    nc.sync.dma_start(out=bo_t, in_=bo[:, :].bitcast(MDT))
    b1_t = consts.tile([P, FFT], F32, name="b1_t", tag="b1_t")
    nc.sync.dma_start(out=b1_t, in_=b1[:].rearrange("(a p) -> p a", p=P))
    b2_t = consts.tile([P, D], F32, name="b2_t", tag="b2_t")
    nc.gpsimd.dma_start(out=b2_t, in_=b2[:, :].to_broadcast((P, D)))

    _layernorm_tiles(nc, small, xraw, ST, eps_t, outs=xn, act_frac=0.0)

    # ---------------- stage B: transpose xn -> xnT [768, 2048] --------------
    cm_aU, aUp = open_pool("aU", DT_N, side="right")
    attnUT = [aUp.tile([P, ROWS], MDT, name="attnUT", tag="attnUT", bufs=DT_N)
              for _ in range(DT_N)]
    cm_xnT, xnTp = open_pool("xnT", DT_N, side="right")
    xnT = [xnTp.tile([P, T], BF, name="xnT", tag="xnT", bufs=DT_N)
           for _ in range(DT_N)]
    cm_tps, tps = open_pool("tps", 4, space="PSUM")
    for k in range(DT_N):
        for i in range(ST):
            pt = tps.tile([P, P], MDT, name="tp", tag="tp", bufs=4)
            nc.tensor.transpose(pt, xn[i][:, k * P:(k + 1) * P], identity_r)
            if (k * ST + i) % 2 == 0:
                nc.vector.tensor_copy(out=xnT[k][:, i * P:(i + 1) * P], in_=pt)
            else:
                nc.scalar.copy(out=xnT[k][:, i * P:(i + 1) * P], in_=pt)
    cm_tps.__exit__(None, None, None)
    cm_xraw.__exit__(None, None, None)
    cm_xn.__exit__(None, None, None)

    # ------------- stages C+D: fused QKV projections + attention ------------
    # KT[k] rows are head-size rows for heads (2k, 2k+1); columns are keys.
    # Only KT[0]/QT[0] are produced up front.  All V tiles are produced
    # interleaved with pair 0's score matmuls; KT[kp+1]/QT[kp+1] are produced
    # during pair kp's steps.  attnV for pair kp-1 interleaves with scores of
    # pair kp so the PE never waits on the ACT exp stream.
    cm_KT, KTp = open_pool("KT", DT_N)
    KT = [KTp.tile([P, T], BF, name="KT", tag="KT", bufs=DT_N)
          for _ in range(DT_N)]
    cm_QT, QTp = open_pool("QT", DT_N)
    QT = [QTp.tile([P, ROWS], BF, name="QT", tag="QT", bufs=DT_N)
          for _ in range(DT_N)]
    cm_V, Vp = open_pool("V", ST)
    V_ext = [Vp.tile([P, H, HS + 1], BF, name="V_ext", tag="V_ext", bufs=ST)
             for _ in range(ST)]

    cm_wk, wkp = open_pool("wkp", 3 * DT_N, side="right")
    wk_t, wq_t, wv_t = [], [], []
    for w, lst in ((wk, wk_t), (wq, wq_t), (wv, wv_t)):
        for kk in range(DT_N):
            t = wkp.tile([P, D], BF, name="w_t", tag="w_t", bufs=3 * DT_N)
            (nc.sync if kk % 2 == 0 else nc.scalar).dma_start(
                out=t, in_=w[kk * P:(kk + 1) * P, :])
            lst.append(t)

    cm_exp, expp = open_pool("expp", 26, side="right")
    cm_dn, dnp = open_pool("dnp", 3, side="right")
    cm_mmps, mmps = open_pool("mmps", 2, space="PSUM")
    cm_scps, scps = open_pool("scps", 2, space="PSUM")
    cm_avps, avps = open_pool("avps", 2, space="PSUM")

    def emit_kt(k, n):
        ps = mmps.tile([P, 512], F32, name="mmps", tag="mmps", bufs=2)
        for kk in range(DT_N):
            _mm(nc, ps, wk_t[kk][:, k * P:(k + 1) * P],
                xnT[kk][:, n * 512:(n + 1) * 512], start=(kk == 0),
                stop=(kk == DT_N - 1 and "k" in skip_bias))
        if "k" not in skip_bias:
            _mm(nc, ps, bk_t[:, k * P:(k + 1) * P], ones_row,
                start=False, stop=True)
        nc.vector.tensor_copy(out=KT[k][:, n * 512:(n + 1) * 512], in_=ps)

    def emit_qt(k):
        ps = mmps.tile([P, 512], F32, name="mmps", tag="mmps", bufs=2)
        for kk in range(DT_N):
            _mm(nc, ps, wq_t[kk][:, k * P:(k + 1) * P],
                xnT[kk][:, 0:ROWS], start=(kk == 0),
                stop=(kk == DT_N - 1 and "q" in skip_bias))
        if "q" not in skip_bias:
            _mm(nc, ps, bq_t[:, k * P:(k + 1) * P], ones_row[:, 0:ROWS],
                start=False, stop=True)
        nc.vector.tensor_copy(out=QT[k], in_=ps)

    def emit_v(i):
        for half in range(2):
            ps = mmps.tile([P, 384], F32, name="mmps", tag="mmps", bufs=2)
            for kk in range(DT_N):
                _mm(nc, ps, xnT[kk][:, i * P:(i + 1) * P],
                    wv_t[kk][:, half * 384:(half + 1) * 384],
                    start=(kk == 0),
                    stop=(kk == DT_N - 1 and "v" in skip_bias))
            if "v" not in skip_bias:
                _mm(nc, ps, onesfull, bv_t[:, half * 384:(half + 1) * 384],
                    start=False, stop=True)
            nc.vector.tensor_scalar_mul(
                out=V_ext[i][:, half * 6:(half + 1) * 6, 0:HS],
                in0=ps.rearrange("p (h v) -> p h v", h=6),
                scalar1=mask_all[:, i:i + 1])
        nc.vector.tensor_scalar_mul(out=V_ext[i][:, :, HS:HS + 1],
                                    in0=ones_col[:, 0:H].unsqueeze(2),
                                    scalar1=mask_all[:, i:i + 1])

    ets = {}
    avs = {}

    def emit_scores_step(kp, jj, et):
        ps0 = scps.tile([P, 2, 512], F32, name="scps", tag="scps", bufs=2)
        ps1 = scps.tile([P, 2, 512], F32, name="scps", tag="scps", bufs=2)
        for u in range(2):
            j = jj * 2 + u
            _mm(nc, ps0[:, u, :], KT[kp][0:64, j * P:(j + 1) * P],
                QT[kp][0:64, :], start=True, stop=True)
            _mm(nc, ps1[:, u, :], KT[kp][64:128, j * P:(j + 1) * P],
                QT[kp][64:128, :], start=True, stop=True)
        e0 = expp.tile([P, 2, 512], BF, name="expT", tag="expT", bufs=26)
        e1 = expp.tile([P, 2, 512], BF, name="expT", tag="expT", bufs=26)
        nc.scalar.activation(out=e0, in_=ps0,
                             func=mybir.ActivationFunctionType.Exp)
        nc.scalar.activation(out=e1, in_=ps1,
                             func=mybir.ActivationFunctionType.Exp)
        et.append((e0, e1))

    def emit_attnv_step(kp, jj):
        av0, av1 = avs[kp]
        et = ets[kp]
        for u in range(2):
            j = jj * 2 + u
            _mm(nc, av0, V_ext[j][:, 2 * kp, :], et[jj][0][:, u, :],
                start=(j == 0), stop=(j == ST - 1))
            _mm(nc, av1, V_ext[j][:, 2 * kp + 1, :], et[jj][1][:, u, :],
                start=(j == 0), stop=(j == ST - 1))

    def finish_pair(kp):
        d_pair = []
        for half, av in ((0, avs[kp][0]), (1, avs[kp][1])):
            nc.vector.tensor_copy(out=attnUT[kp][half * 64:half * 64 + 64, :],
                                  in_=av[0:64, :])
            d_sb = dnp.tile([1, 512], MDT, name="d_sb", tag="d_sb", bufs=3)
            nc.vector.tensor_copy(out=d_sb, in_=av[64:65, :])
            d_pair.append(d_sb)
        # R broadcast reuses a scores psum slot; reciprocal on 128 lanes
        rp_t = scps.tile([P, 2, 512], F32, name="scps", tag="scps", bufs=2)
        rp = rp_t[:, 0, :]
        _mm(nc, rp, onesA, d_pair[0], start=True, stop=False)
        _mm(nc, rp, onesB, d_pair[1], start=False, stop=True)
        rr = dnp.tile([P, 512], F32, name="rrec", tag="rrec", bufs=2)
        nc.vector.reciprocal(out=rr, in_=rp)
        nc.vector.tensor_mul(out=attnUT[kp], in0=attnUT[kp], in1=rr)

    emit_kt(0, 0), emit_kt(0, 1), emit_kt(0, 2), emit_kt(0, 3)
    emit_qt(0)
    for kp in range(HP + 1):
        if kp < HP:
            ets[kp] = []
        if kp >= 1:
            avs[kp - 1] = (
                avps.tile([HS + 1, 512], F32, name="avps", tag="avps", bufs=2),
                avps.tile([HS + 1, 512], F32, name="avps", tag="avps", bufs=2))
        for jj in range(ST // 2):
            if kp < HP:
                emit_scores_step(kp, jj, ets[kp])
            if kp >= 1:
                emit_attnv_step(kp - 1, jj)
            if kp == 0:
                emit_v(2 * jj)
                emit_v(2 * jj + 1)
                if jj < 4:
                    emit_kt(1, jj)
                elif jj == 4:
                    emit_qt(1)
            elif kp + 1 < HP:
                if jj < 4:
                    emit_kt(kp + 1, jj)
                elif jj == 4:
                    emit_qt(kp + 1)
        if kp >= 1:
            finish_pair(kp - 1)
            del ets[kp - 1]
    cm_avps.__exit__(None, None, None)
    cm_scps.__exit__(None, None, None)
    cm_mmps.__exit__(None, None, None)
    cm_dn.__exit__(None, None, None)
    cm_exp.__exit__(None, None, None)
    cm_wk.__exit__(None, None, None)
    cm_xnT.__exit__(None, None, None)
    cm_V.__exit__(None, None, None)
    cm_QT.__exit__(None, None, None)
    cm_KT.__exit__(None, None, None)

    # ---------------- stage E: out-projection + residual --------------------
    cm_y1, y1p = open_pool("y1", 2 * QT_N)
    y1 = [y1p.tile([P, D], F32, name="y1", tag="y1y", bufs=2 * QT_N)
          for _ in range(QT_N)]
    y_acc = [y1p.tile([P, D], F32, name="yacc", tag="y1y", bufs=2 * QT_N)
             for _ in range(QT_N)]
    cm_xsl, xslp = open_pool("xslp", QT_N)
    xsl_t = []
    for i in range(QT_N):
        t = xslp.tile([P, D], F32, name="xsl", tag="xsl", bufs=QT_N)
        (nc.sync if i % 2 == 0 else nc.scalar).dma_start(
            out=t, in_=xsl[i * P:(i + 1) * P, :])
        xsl_t.append(t)
    cm_wo, wop = open_pool("wop", DT_N, side="right")
    wo_t = []
    for kk in range(DT_N):
        t = wop.tile([P, D], MDT, name="wo_t", tag="wo_t", bufs=DT_N)
        (nc.sync if kk % 2 == 0 else nc.scalar).dma_start(
            out=t, in_=wo[kk * P:(kk + 1) * P, :].bitcast(MDT))
        wo_t.append(t)
    cm_pps, pps = open_pool("pps", 4, space="PSUM")
    for tm in range(QT_N):
        for n in range(2):
            ps = pps.tile([P, 384], F32, name="pps", tag="pps", bufs=4)
            for kk in range(DT_N):
                _mm(nc, ps, attnUT[kk][:, tm * P:(tm + 1) * P],
                    wo_t[kk][:, n * 384:(n + 1) * 384],
                    start=(kk == 0),
                    stop=(kk == DT_N - 1 and "o" in skip_bias))
            if "o" not in skip_bias:
                _mm(nc, ps, onesfull, bo_t[:, n * 384:(n + 1) * 384],
                    start=False, stop=True)
            nc.vector.tensor_add(out=y1[tm][:, n * 384:(n + 1) * 384],
                                 in0=ps, in1=xsl_t[tm][:, n * 384:(n + 1) * 384])
    cm_pps.__exit__(None, None, None)
    cm_wo.__exit__(None, None, None)
    cm_aU.__exit__(None, None, None)
    cm_xsl.__exit__(None, None, None)

    # ---------------- stage F: LN2 + transpose ------------------------------
    cm_y2T, y2Tp = open_pool("y2T", DT_N, side="right")
    y2nT = [y2Tp.tile([P, ROWS], MDT, name="y2nT", tag="y2nT", bufs=DT_N)
            for _ in range(DT_N)]
    cm_y2, y2p = open_pool("y2", QT_N)
    y2n = [y2p.tile([P, D], MDT, name="y2n", tag="y2n", bufs=QT_N)
           for _ in range(QT_N)]
    _layernorm_tiles(nc, small, y1, QT_N, eps_t, outs=y2n)
    cm_tps2, tps2 = open_pool("tps2", 4, space="PSUM")
    for k in range(DT_N):
        for tm in range(QT_N):
            pt = tps2.tile([P, P], MDT, name="tp2", tag="tp2", bufs=4)
            nc.tensor.transpose(pt, y2n[tm][:, k * P:(k + 1) * P], identity_r)
            if (k * QT_N + tm) % 2 == 0:
                nc.vector.tensor_copy(out=y2nT[k][:, tm * P:(tm + 1) * P], in_=pt)
            else:
                nc.scalar.copy(out=y2nT[k][:, tm * P:(tm + 1) * P], in_=pt)
    cm_tps2.__exit__(None, None, None)
    cm_y2.__exit__(None, None, None)

    # ---------------- stage G: FFN1 -> ffhT (feature-major, bias+relu) ------
    cm_ffh, ffhp = open_pool("ffh", FFT)
    cm_w1, w1p = open_pool("w1p", 4, side="right")
    cm_fps, fps = open_pool("fps", 3, space="PSUM")
    ffhT = []
    for m in range(FFT):
        wt = w1p.tile([P, DT_N, P], MDT, name="w1c", tag="w1c", bufs=4)
        (nc.sync if m % 2 == 0 else nc.scalar).dma_start(
            out=wt, in_=w1[:, m * P:(m + 1) * P].rearrange("(ko p) f -> p ko f", p=P).bitcast(MDT))
        ps = fps.tile([P, 512], F32, name="fps", tag="fps", bufs=3)
        for kk in range(DT_N):
            _mm(nc, ps, wt[:, kk, :], y2nT[kk],
                start=(kk == 0), stop=(kk == DT_N - 1))
        ft = ffhp.tile([P, ROWS], MDT, name="ffhT", tag="ffhT", bufs=FFT)
        nc.vector.tensor_scalar(out=ft, in0=ps,
                                scalar1=b1_t[:, m:m + 1], scalar2=0.0,
                                op0=mybir.AluOpType.add,
                                op1=mybir.AluOpType.max)
        ffhT.append(ft)
    cm_fps.__exit__(None, None, None)
    cm_w1.__exit__(None, None, None)
    cm_y2T.__exit__(None, None, None)

    # ---------------- stage H: FFN2 + residual (SBUF accumulation) ----------
    cm_w2, w2p = open_pool("w2p", 10, side="right")
    cm_cps, cps = open_pool("cps", 3, space="PSUM")
    for tm in range(QT_N):
        nc.vector.tensor_add(out=y_acc[tm], in0=y1[tm], in1=b2_t)
    KC = 8
    for chunk in range(FFT // KC):
        w2_t = []
        for kk in range(KC):
            m = chunk * KC + kk
            t = w2p.tile([P, D], MDT, name="w2_t", tag="w2_t", bufs=10)
            (nc.sync if m % 2 == 0 else nc.scalar).dma_start(
                out=t, in_=w2[m * P:(m + 1) * P, :].bitcast(MDT))
            w2_t.append(t)
        for tm in range(QT_N):
            for n in range(2):
                ps = cps.tile([P, 384], F32, name="cps", tag="cps", bufs=3)
                for kk in range(KC):
                    m = chunk * KC + kk
                    _mm(nc, ps, ffhT[m][:, tm * P:(tm + 1) * P],
                        w2_t[kk][:, n * 384:(n + 1) * 384],
                        start=(kk == 0), stop=(kk == KC - 1))
                nc.vector.tensor_add(out=y_acc[tm][:, n * 384:(n + 1) * 384],
                                     in0=y_acc[tm][:, n * 384:(n + 1) * 384],
                                     in1=ps)
    for tm in range(QT_N):
        nc.sync.dma_start(out=y[tm * P:(tm + 1) * P, :], in_=y_acc[tm])
    cm_cps.__exit__(None, None, None)
    cm_w2.__exit__(None, None, None)
    cm_ffh.__exit__(None, None, None)
    cm_y1.__exit__(None, None, None)
    cm_small.__exit__(None, None, None)
    cm_const.__exit__(None, None, None)


# ---------------------------------------------------------------------------
# host side
# ---------------------------------------------------------------------------
_NC_CACHE = {}


def _bias_flags(in_maps):
    m = in_maps[0]
    return frozenset(f for f in "qkvo" if not np.any(m["b" + f]))


def _get_nc(skip_bias=frozenset()):
    key = (MM_DT, skip_bias)
    if key not in _NC_CACHE:
        _NC_CACHE[key] = build_nc(skip_bias=skip_bias)
    return _NC_CACHE[key]


def _make_in_maps(inputs):
    return _prep_inputs(**{k: np.asarray(v) for k, v in inputs.items()})


def _prep_inputs(x, attn_mask, Wq, Wk, Wv, Wo, bo, ln1_g, ln1_b, ln2_g, ln2_b,
                 W1, b1, W2, b2):
    x = np.asarray(x, dtype=np.float32)
    attn_mask = np.asarray(attn_mask)
    f64 = np.float64
    g1 = np.asarray(ln1_g, f64)
    lb1 = np.asarray(ln1_b, f64)
    g2 = np.asarray(ln2_g, f64)
    lb2 = np.asarray(ln2_b, f64)
    Wq = np.asarray(Wq, f64)
    Wk = np.asarray(Wk, f64)
    Wv = np.asarray(Wv, f64)
    s = float(D) ** -0.5

    import ml_dtypes
    wq_e = ((g1[:, None] * Wq) * s).astype(ml_dtypes.bfloat16)
    bq_e = ((lb1 @ Wq) * s).astype(np.float32)[None, :]
    wk_e = (g1[:, None] * Wk).astype(ml_dtypes.bfloat16)
    bk_e = (lb1 @ Wk).astype(np.float32)[None, :]
    wv_e = (g1[:, None] * Wv).astype(ml_dtypes.bfloat16)
    bv_e = (lb1 @ Wv).astype(np.float32)[None, :]
    wo_e = np.asarray(Wo, np.float32)
    bo_e = np.asarray(bo, np.float32)[None, :]
    W1_64 = np.asarray(W1, f64)
    w1_e = (g2[:, None] * W1_64).astype(np.float32)
    b1_e = (np.asarray(b1, f64) + lb2 @ W1_64).astype(np.float32)
    w2_e = np.asarray(W2, np.float32)
    b2_e = np.asarray(b2, np.float32)[None, :]

    maskf = attn_mask.astype(np.float32)

    in_maps = []
    for c in range(NCORES):
        b = c // CPB
        r0 = (c % CPB) * ROWS
        in_maps.append({
            "xb": np.ascontiguousarray(np.roll(x[b], -r0, axis=0)),
            "xsl": np.ascontiguousarray(x[b, r0:r0 + ROWS]),
            "maskf": np.ascontiguousarray(np.roll(maskf[b], -r0)[:, None]),
            "wq": wq_e, "wk": wk_e, "wv": wv_e, "wo": wo_e,
            "w1": w1_e, "w2": w2_e,
            "bq": bq_e, "bk": bk_e, "bv": bv_e, "bo": bo_e,
            "b1": b1_e, "b2": b2_e,
        })
    return in_maps


def kernel(**inputs):
    in_maps = _make_in_maps(inputs)
    nc = _get_nc(_bias_flags(in_maps))
    res = run_bass_kernel_spmd(nc, in_maps, list(range(NCORES)))

    out = np.empty((B, T, D), dtype=np.float32)
    for c in range(NCORES):
        b = c // CPB
        r0 = (c % CPB) * ROWS
        out[b, r0:r0 + ROWS] = res.results[c]["y"]
    return out



# revision 3
# speedup vs baseline: 196.9888x; 196.9888x over previous
"""Trainium2 Bass kernel for a dense transformer block (B=2, T=2048, D=768, H=12).

Sharding: 8 cores, each owns 512 contiguous tokens of one batch element
(4 cores per batch).  Each core receives its batch's full token stream
(rotated so its own 512 query rows come first), computes K/V for all 2048
keys of that batch locally (no cross-core communication), and attention +
FFN for its own 512 rows.  Host gathers the 8 row-slices.

LayerNorm affine params and the attention scale are folded into the weight
matrices on the host, so the device only computes pure (x-mean)*rstd
normalizations.
"""

import os
import numpy as np

import concourse.bass as bass
import concourse.tile as tile
from concourse import bacc, mybir
from concourse.bass_utils import run_bass_kernel_spmd
from concourse.masks import make_identity

F32 = mybir.dt.float32
BF = mybir.dt.bfloat16

D = 768
H = 12
HS = 64
B = 2
T = 2048
P = 128
NCORES = 8
CPB = NCORES // B          # cores per batch
ROWS = T // CPB            # 512 query rows per core
ST = T // P                # 16 key tiles
QT_N = ROWS // P           # 4 query tiles
DT_N = D // P              # 6 feature tiles
FF = 4 * D                 # 3072
FFT = FF // P              # 24 ff tiles
HP = H // 2                # 6 head pairs
EPS = 1e-5

# matmul input dtype: float32 (exact, 4 cyc/row) or float32r (~1 cyc/row)
MM_DT = {
    "f32": mybir.dt.float32,
    "f32r": mybir.dt.float32r,
}[os.environ.get("KERNEL_MM_DT", "f32r")]


MDT = MM_DT  # dtype of tiles that feed matmuls (producers round to f32r)


def _mm(nc, out, lhsT, rhs, **kw):
    nc.tensor.matmul(out, lhsT, rhs, **kw)


def build_nc(reps=None, skip_bias=(), loop=False):
    nc = bacc.Bacc("TRN2", target_bir_lowering=False, debug=False, num_devices=NCORES)

    xb = nc.declare_dram_parameter("xb", [T, D], F32, isOutput=False)
    xsl = nc.declare_dram_parameter("xsl", [ROWS, D], F32, isOutput=False)
    maskf = nc.declare_dram_parameter("maskf", [T, 1], F32, isOutput=False)
    wq = nc.declare_dram_parameter("wq", [D, D], BF, isOutput=False)
    wk = nc.declare_dram_parameter("wk", [D, D], BF, isOutput=False)
    wv = nc.declare_dram_parameter("wv", [D, D], BF, isOutput=False)
    wo = nc.declare_dram_parameter("wo", [D, D], F32, isOutput=False)
    w1 = nc.declare_dram_parameter("w1", [D, FF], F32, isOutput=False)
    w2 = nc.declare_dram_parameter("w2", [FF, D], F32, isOutput=False)
    bq = nc.declare_dram_parameter("bq", [1, D], F32, isOutput=False)
    bk = nc.declare_dram_parameter("bk", [1, D], F32, isOutput=False)
    bv = nc.declare_dram_parameter("bv", [1, D], F32, isOutput=False)
    bo = nc.declare_dram_parameter("bo", [1, D], F32, isOutput=False)
    b1 = nc.declare_dram_parameter("b1", [FF], F32, isOutput=False)
    b2 = nc.declare_dram_parameter("b2", [1, D], F32, isOutput=False)
    y = nc.declare_dram_parameter("y", [ROWS, D], F32, isOutput=True)

    if reps is None:
        reps = int(os.environ.get("KERNEL_REPS", "1"))
    with tile.TileContext(nc) as tc, \
            nc.allow_low_precision(reason="f32r-rounded matmul operands"):
        if loop:
            # hardware loop: body emitted once, executed `reps` times
            with tc.For_i(0, reps):
                _emit(nc, tc, xb, xsl, maskf, wq, wk, wv, wo, w1, w2,
                      bq, bk, bv, bo, b1, b2, y, skip_bias=frozenset(skip_bias))
        else:
            for r in range(reps):
                if r:
                    tc.strict_bb_all_engine_barrier()
                _emit(nc, tc, xb, xsl, maskf, wq, wk, wv, wo, w1, w2,
                      bq, bk, bv, bo, b1, b2, y, skip_bias=frozenset(skip_bias))
    nc.compile()
    return nc


def _layernorm_tiles(nc, pool, tiles, n, eps_t, outs=None, act_frac=0.0):
    """(x-mean)*rstd over `tiles[:n]` ([P, D] token-major), in place unless
    `outs` supplies destination tiles.  A leading fraction of the tiles is
    handled on the ACT engine (Square/Identity passes with accum_out) so DVE
    and ACT split the work."""
    n_act = int(n * act_frac)
    for i in range(n):
        xt = tiles[i]
        out_t = xt if outs is None else outs[i]
        std = pool.tile([P, 1], F32, name="lnstd", tag="lnstd", bufs=4)
        if i < n_act:
            scr = pool.tile([P, D], F32, name="lnscr", tag="lnscr", bufs=1)
            sums = pool.tile([P, 2], F32, name="lnsums", tag="lnsums", bufs=4)
            nc.scalar.activation(out=scr, in_=xt,
                                 func=mybir.ActivationFunctionType.Square,
                                 accum_out=sums[:, 1:2])
            nc.scalar.activation(out=scr, in_=xt,
                                 func=mybir.ActivationFunctionType.Identity,
                                 accum_out=sums[:, 0:1])
            mv = pool.tile([P, 2], F32, name="lnmv", tag="lnmv", bufs=4)
            # mean = sum/D ; var = sumsq/D - mean^2
            nc.vector.tensor_scalar_mul(out=mv, in0=sums, scalar1=1.0 / D)
            m2 = pool.tile([P, 1], F32, name="lnm2", tag="lnm2", bufs=4)
            nc.vector.tensor_mul(out=m2, in0=mv[:, 0:1], in1=mv[:, 0:1])
            nc.vector.tensor_sub(out=mv[:, 1:2], in0=mv[:, 1:2], in1=m2)
            nc.scalar.activation(out=std, in_=mv[:, 1:2],
                                 func=mybir.ActivationFunctionType.Sqrt,
                                 bias=eps_t)
            nc.vector.reciprocal(out=std, in_=std)
            nmr = pool.tile([P, 1], F32, name="lnnmr", tag="lnnmr", bufs=4)
            nc.vector.scalar_tensor_tensor(out=nmr, in0=mv[:, 0:1],
                                           scalar=-1.0, in1=std,
                                           op0=mybir.AluOpType.mult,
                                           op1=mybir.AluOpType.mult)
            nc.scalar.activation(out=out_t, in_=xt,
                                 func=mybir.ActivationFunctionType.Identity,
                                 bias=nmr, scale=std)
        else:
            stats = pool.tile([P, 3, 6], F32, name="lnstats", tag="lnstats",
                              bufs=4)
            for g in range(3):
                nc.vector.bn_stats(out=stats[:, g, :],
                                   in_=xt[:, g * 256:(g + 1) * 256])
            mv = pool.tile([P, 2], F32, name="lnmv", tag="lnmv", bufs=4)
            nc.vector.bn_aggr(out=mv, in_=stats)
            nc.scalar.activation(out=std, in_=mv[:, 1:2],
                                 func=mybir.ActivationFunctionType.Sqrt,
                                 bias=eps_t)
            nc.vector.reciprocal(out=std, in_=std)
            nc.vector.tensor_scalar(out=out_t, in0=xt,
                                    scalar1=mv[:, 0:1], scalar2=std,
                                    op0=mybir.AluOpType.subtract,
                                    op1=mybir.AluOpType.mult)


def _emit(nc, tc, xb, xsl, maskf, wq, wk, wv, wo, w1, w2,
          bq, bk, bv, bo, b1, b2, y, skip_bias=frozenset()):
    # Pools are released in strict LIFO order per (space, side) stack; the
    # open/close sequence below alternates sides so that overlapping
    # non-nested lifetimes land on different stacks.
    def open_pool(name, bufs, space="SBUF", side=None):
        cm = tc.tile_pool(name=name, bufs=bufs, space=space, side=side)
        return cm, cm.__enter__()

    # ---------------- constants ----------------
    cm_const, consts = open_pool("consts", 1)
    identity = consts.tile([P, P], F32, name="identity", tag="identity")
    make_identity(nc, identity)
    identity_r = consts.tile([P, P], MDT, name="identity_r", tag="identity_r")
    nc.vector.tensor_copy(out=identity_r, in_=identity)
    ones_st = consts.tile([1, 512 + 3 * P], F32, name="ones_st", tag="ones_st")
    nc.vector.memset(ones_st[:, 0:512 + P], 1.0)
    nc.vector.memset(ones_st[:, 512 + P:], 0.0)
    nc.vector.memset(ones_st[:, 512 + P:512 + P + 64], 1.0)
    nc.vector.memset(ones_st[:, 512 + 2 * P + 64:], 1.0)
    ones_row = consts.tile([1, 512], MDT, name="ones_row", tag="ones_row")
    nc.vector.tensor_copy(out=ones_row, in_=ones_st[:, 0:512])
    onesfull = consts.tile([1, P], MDT, name="onesfull", tag="onesfull")
    nc.vector.tensor_copy(out=onesfull, in_=ones_st[:, 512:512 + P])
    onesA = consts.tile([1, P], MDT, name="onesA", tag="onesA")
    nc.vector.tensor_copy(out=onesA, in_=ones_st[:, 512 + P:512 + 2 * P])
    onesB = consts.tile([1, P], MDT, name="onesB", tag="onesB")
    nc.vector.tensor_copy(out=onesB, in_=ones_st[:, 512 + 2 * P:512 + 3 * P])
    eps_t = consts.tile([P, 1], F32, name="eps_t", tag="eps_t")
    nc.vector.memset(eps_t, EPS)
    ones_col = consts.tile([P, H], F32, name="ones_col", tag="ones_col")
    nc.vector.memset(ones_col, 1.0)

    cm_small, small = open_pool("small", 4)

    # ---------------- stage A: load x batch + LN1 (in place) ----------------
    # x tiles are loaded first (alternating HWDGE queues) so LN + transposes
    # start as early as possible; small const loads follow.
    cm_xn, xnp = open_pool("xn", ST)
    cm_xraw, xrawp = open_pool("xraw", 6)
    xn = [xnp.tile([P, D], MDT, name="xn", tag="xn", bufs=ST)
          for _ in range(ST)]
    xraw = []
    for i in range(ST):
        t = xrawp.tile([P, D], F32, name="xraw", tag="xraw", bufs=6)
        eng = nc.sync if i % 2 == 0 else nc.scalar
        eng.dma_start(out=t, in_=xb[i * P:(i + 1) * P, :])
        xraw.append(t)

    mask_all = consts.tile([P, ST], F32, name="mask_all", tag="mask_all")
    nc.sync.dma_start(out=mask_all, in_=maskf[:, :].rearrange("(n p) o -> p (n o)", p=P))
    bq_t = consts.tile([1, D], MDT, name="bq_t", tag="bq_t")
    nc.sync.dma_start(out=bq_t, in_=bq[:, :].bitcast(MDT))
    bk_t = consts.tile([1, D], MDT, name="bk_t", tag="bk_t")
    nc.sync.dma_start(out=bk_t, in_=bk[:, :].bitcast(MDT))
    bv_t = consts.tile([1, D], MDT, name="bv_t", tag="bv_t")
    nc.sync.dma_start(out=bv_t, in_=bv[:, :].bitcast(MDT))
    bo_t = consts.tile([1, D], MDT, name="bo_t", tag="bo_t")
    nc.sync.dma_start(out=bo_t, in_=bo[:, :].bitcast(MDT))
    b1_t = consts.tile([P, FFT], F32, name="b1_t", tag="b1_t")
    nc.sync.dma_start(out=b1_t, in_=b1[:].rearrange("(a p) -> p a", p=P))
    b2_t = consts.tile([P, D], F32, name="b2_t", tag="b2_t")
    nc.gpsimd.dma_start(out=b2_t, in_=b2[:, :].to_broadcast((P, D)))

    _layernorm_tiles(nc, small, xraw, ST, eps_t, outs=xn, act_frac=0.0)

    # ---------------- stage B: transpose xn -> xnT [768, 2048] --------------
    cm_aU, aUp = open_pool("aU", DT_N, side="right")
    attnUT = [aUp.tile([P, ROWS], MDT, name="attnUT", tag="attnUT", bufs=DT_N)
              for _ in range(DT_N)]
    cm_xnT, xnTp = open_pool("xnT", DT_N, side="right")
    xnT = [xnTp.tile([P, T], BF, name="xnT", tag="xnT", bufs=DT_N)
           for _ in range(DT_N)]
    cm_tps, tps = open_pool("tps", 4, space="PSUM")
    for k in range(DT_N):
        for i in range(ST):
            pt = tps.tile([P, P], MDT, name="tp", tag="tp", bufs=4)
            nc.tensor.transpose(pt, xn[i][:, k * P:(k + 1) * P], identity_r)
            if (k * ST + i) % 2 == 0:
                nc.vector.tensor_copy(out=xnT[k][:, i * P:(i + 1) * P], in_=pt)
            else:
                nc.scalar.copy(out=xnT[k][:, i * P:(i + 1) * P], in_=pt)
    cm_tps.__exit__(None, None, None)
    cm_xraw.__exit__(None, None, None)
    cm_xn.__exit__(None, None, None)

    # ------------- stages C+D: fused QKV projections + attention ------------
    # KT[k] rows are head-size rows for heads (2k, 2k+1); columns are keys.
    # Only KT[0]/QT[0] are produced up front.  All V tiles are produced
    # interleaved with pair 0's score matmuls; KT[kp+1]/QT[kp+1] are produced
    # during pair kp's steps.  attnV for pair kp-1 interleaves with scores of
    # pair kp so the PE never waits on the ACT exp stream.
    cm_KT, KTp = open_pool("KT", DT_N)
    KT = [KTp.tile([P, T], BF, name="KT", tag="KT", bufs=DT_N)
          for _ in range(DT_N)]
    cm_QT, QTp = open_pool("QT", DT_N)
    QT = [QTp.tile([P, ROWS], BF, name="QT", tag="QT", bufs=DT_N)
          for _ in range(DT_N)]
    cm_V, Vp = open_pool("V", ST)
    V_ext = [Vp.tile([P, H, HS + 1], BF, name="V_ext", tag="V_ext", bufs=ST)
             for _ in range(ST)]

    cm_wk, wkp = open_pool("wkp", 3 * DT_N, side="right")
    wk_t, wq_t, wv_t = [], [], []
    for w, lst in ((wk, wk_t), (wq, wq_t), (wv, wv_t)):
        for kk in range(DT_N):
            t = wkp.tile([P, D], BF, name="w_t", tag="w_t", bufs=3 * DT_N)
            (nc.sync if kk % 2 == 0 else nc.scalar).dma_start(
                out=t, in_=w[kk * P:(kk + 1) * P, :])
            lst.append(t)

    cm_exp, expp = open_pool("expp", 26, side="right")
    cm_dn, dnp = open_pool("dnp", 3, side="right")
    cm_mmps, mmps = open_pool("mmps", 2, space="PSUM")
    cm_scps, scps = open_pool("scps", 2, space="PSUM")
    cm_avps, avps = open_pool("avps", 2, space="PSUM")

    def emit_kt(k, n):
        ps = mmps.tile([P, 512], F32, name="mmps", tag="mmps", bufs=2)
        for kk in range(DT_N):
            _mm(nc, ps, wk_t[kk][:, k * P:(k + 1) * P],
                xnT[kk][:, n * 512:(n + 1) * 512], start=(kk == 0),
                stop=(kk == DT_N - 1 and "k" in skip_bias))
        if "k" not in skip_bias:
            _mm(nc, ps, bk_t[:, k * P:(k + 1) * P], ones_row,
                start=False, stop=True)
        nc.vector.tensor_copy(out=KT[k][:, n * 512:(n + 1) * 512], in_=ps)

    def emit_qt(k):
        ps = mmps.tile([P, 512], F32, name="mmps", tag="mmps", bufs=2)
        for kk in range(DT_N):
            _mm(nc, ps, wq_t[kk][:, k * P:(k + 1) * P],
                xnT[kk][:, 0:ROWS], start=(kk == 0),
                stop=(kk == DT_N - 1 and "q" in skip_bias))
        if "q" not in skip_bias:
            _mm(nc, ps, bq_t[:, k * P:(k + 1) * P], ones_row[:, 0:ROWS],
                start=False, stop=True)
        nc.vector.tensor_copy(out=QT[k], in_=ps)

    def emit_v(i):
        for half in range(2):
            ps = mmps.tile([P, 384], F32, name="mmps", tag="mmps", bufs=2)
            for kk in range(DT_N):
                _mm(nc, ps, xnT[kk][:, i * P:(i + 1) * P],
                    wv_t[kk][:, half * 384:(half + 1) * 384],
                    start=(kk == 0),
                    stop=(kk == DT_N - 1 and "v" in skip_bias))
            if "v" not in skip_bias:
                _mm(nc, ps, onesfull, bv_t[:, half * 384:(half + 1) * 384],
                    start=False, stop=True)
            nc.vector.tensor_scalar_mul(
                out=V_ext[i][:, half * 6:(half + 1) * 6, 0:HS],
                in0=ps.rearrange("p (h v) -> p h v", h=6),
                scalar1=mask_all[:, i:i + 1])
        nc.vector.tensor_scalar_mul(out=V_ext[i][:, :, HS:HS + 1],
                                    in0=ones_col[:, 0:H].unsqueeze(2),
                                    scalar1=mask_all[:, i:i + 1])

    ets = {}
    avs = {}

    def emit_scores_step(kp, jj, et):
        ps0 = scps.tile([P, 2, 512], F32, name="scps", tag="scps", bufs=2)
        ps1 = scps.tile([P, 2, 512], F32, name="scps", tag="scps", bufs=2)
        for u in range(2):
            j = jj * 2 + u
            _mm(nc, ps0[:, u, :], KT[kp][0:64, j * P:(j + 1) * P],
                QT[kp][0:64, :], start=True, stop=True)
            _mm(nc, ps1[:, u, :], KT[kp][64:128, j * P:(j + 1) * P],
                QT[kp][64:128, :], start=True, stop=True)
        e0 = expp.tile([P, 2, 512], BF, name="expT", tag="expT", bufs=26)
        e1 = expp.tile([P, 2, 512], BF, name="expT", tag="expT", bufs=26)
        nc.scalar.activation(out=e0, in_=ps0,
                             func=mybir.ActivationFunctionType.Exp)
        nc.scalar.activation(out=e1, in_=ps1,
                             func=mybir.ActivationFunctionType.Exp)
        et.append((e0, e1))

    def emit_attnv_step(kp, jj):
        av0, av1 = avs[kp]
        et = ets[kp]
        for u in range(2):
            j = jj * 2 + u
            _mm(nc, av0, V_ext[j][:, 2 * kp, :], et[jj][0][:, u, :],
                start=(j == 0), stop=(j == ST - 1))
            _mm(nc, av1, V_ext[j][:, 2 * kp + 1, :], et[jj][1][:, u, :],
                start=(j == 0), stop=(j == ST - 1))

    def finish_pair(kp):
        d_pair = []
        for half, av in ((0, avs[kp][0]), (1, avs[kp][1])):
            nc.vector.tensor_copy(out=attnUT[kp][half * 64:half * 64 + 64, :],
                                  in_=av[0:64, :])
            d_sb = dnp.tile([1, 512], MDT, name="d_sb", tag="d_sb", bufs=3)
            nc.vector.tensor_copy(out=d_sb, in_=av[64:65, :])
            d_pair.append(d_sb)
        # R broadcast reuses a scores psum slot; reciprocal on 128 lanes
        rp_t = scps.tile([P, 2, 512], F32, name="scps", tag="scps", bufs=2)
        rp = rp_t[:, 0, :]
        _mm(nc, rp, onesA, d_pair[0], start=True, stop=False)
        _mm(nc, rp, onesB, d_pair[1], start=False, stop=True)
        rr = dnp.tile([P, 512], F32, name="rrec", tag="rrec", bufs=2)
        nc.vector.reciprocal(out=rr, in_=rp)
        nc.vector.tensor_mul(out=attnUT[kp], in0=attnUT[kp], in1=rr)

    emit_kt(0, 0), emit_kt(0, 1), emit_kt(0, 2), emit_kt(0, 3)
    emit_qt(0)
    for kp in range(HP + 1):
        if kp < HP:
            ets[kp] = []
        if kp >= 1:
            avs[kp - 1] = (
                avps.tile([HS + 1, 512], F32, name="avps", tag="avps", bufs=2),
                avps.tile([HS + 1, 512], F32, name="avps", tag="avps", bufs=2))
        for jj in range(ST // 2):
            if kp < HP:
                emit_scores_step(kp, jj, ets[kp])
            if kp >= 1:
                emit_attnv_step(kp - 1, jj)
            if kp == 0:
                emit_v(2 * jj)
                emit_v(2 * jj + 1)
                if jj < 4:
                    emit_kt(1, jj)
                elif jj == 4:
                    emit_qt(1)
            elif kp + 1 < HP:
                if jj < 4:
                    emit_kt(kp + 1, jj)
                elif jj == 4:
                    emit_qt(kp + 1)
        if kp >= 1:
            finish_pair(kp - 1)
            del ets[kp - 1]
    cm_avps.__exit__(None, None, None)
    cm_scps.__exit__(None, None, None)
    cm_mmps.__exit__(None, None, None)
    cm_dn.__exit__(None, None, None)
    cm_exp.__exit__(None, None, None)
    cm_wk.__exit__(None, None, None)
    cm_xnT.__exit__(None, None, None)
    cm_V.__exit__(None, None, None)
    cm_QT.__exit__(None, None, None)
    cm_KT.__exit__(None, None, None)

    # ---------------- stage E: out-projection + residual --------------------
    cm_y1, y1p = open_pool("y1", 2 * QT_N)
    y1 = [y1p.tile([P, D], F32, name="y1", tag="y1y", bufs=2 * QT_N)
          for _ in range(QT_N)]
    y_acc = [y1p.tile([P, D], F32, name="yacc", tag="y1y", bufs=2 * QT_N)
             for _ in range(QT_N)]
    cm_xsl, xslp = open_pool("xslp", QT_N)
    xsl_t = []
    for i in range(QT_N):
        t = xslp.tile([P, D], F32, name="xsl", tag="xsl", bufs=QT_N)
        (nc.sync if i % 2 == 0 else nc.scalar).dma_start(
            out=t, in_=xsl[i * P:(i + 1) * P, :])
        xsl_t.append(t)
    cm_wo, wop = open_pool("wop", DT_N, side="right")
    wo_t = []
    for kk in range(DT_N):
        t = wop.tile([P, D], MDT, name="wo_t", tag="wo_t", bufs=DT_N)
        (nc.sync if kk % 2 == 0 else nc.scalar).dma_start(
            out=t, in_=wo[kk * P:(kk + 1) * P, :].bitcast(MDT))
        wo_t.append(t)
    cm_pps, pps = open_pool("pps", 4, space="PSUM")
    for tm in range(QT_N):
        for n in range(2):
            ps = pps.tile([P, 384], F32, name="pps", tag="pps", bufs=4)
            for kk in range(DT_N):
                _mm(nc, ps, attnUT[kk][:, tm * P:(tm + 1) * P],
                    wo_t[kk][:, n * 384:(n + 1) * 384],
                    start=(kk == 0),
                    stop=(kk == DT_N - 1 and "o" in skip_bias))
            if "o" not in skip_bias:
                _mm(nc, ps, onesfull, bo_t[:, n * 384:(n + 1) * 384],
                    start=False, stop=True)
            nc.vector.tensor_add(out=y1[tm][:, n * 384:(n + 1) * 384],
                                 in0=ps, in1=xsl_t[tm][:, n * 384:(n + 1) * 384])
    cm_pps.__exit__(None, None, None)
    cm_wo.__exit__(None, None, None)
    cm_aU.__exit__(None, None, None)
    cm_xsl.__exit__(None, None, None)

    # ---------------- stage F: LN2 + transpose ------------------------------
    cm_y2T, y2Tp = open_pool("y2T", DT_N, side="right")
    y2nT = [y2Tp.tile([P, ROWS], MDT, name="y2nT", tag="y2nT", bufs=DT_N)
            for _ in range(DT_N)]
    cm_y2, y2p = open_pool("y2", QT_N)
    y2n = [y2p.tile([P, D], MDT, name="y2n", tag="y2n", bufs=QT_N)
           for _ in range(QT_N)]
    _layernorm_tiles(nc, small, y1, QT_N, eps_t, outs=y2n)
    cm_tps2, tps2 = open_pool("tps2", 4, space="PSUM")
    for k in range(DT_N):
        for tm in range(QT_N):
            pt = tps2.tile([P, P], MDT, name="tp2", tag="tp2", bufs=4)
            nc.tensor.transpose(pt, y2n[tm][:, k * P:(k + 1) * P], identity_r)
            if (k * QT_N + tm) % 2 == 0:
                nc.vector.tensor_copy(out=y2nT[k][:, tm * P:(tm + 1) * P], in_=pt)
            else:
                nc.scalar.copy(out=y2nT[k][:, tm * P:(tm + 1) * P], in_=pt)
    cm_tps2.__exit__(None, None, None)
    cm_y2.__exit__(None, None, None)

    # ---------------- stage G: FFN1 -> ffhT (feature-major, bias+relu) ------
    cm_ffh, ffhp = open_pool("ffh", FFT)
    cm_w1, w1p = open_pool("w1p", 4, side="right")
    cm_fps, fps = open_pool("fps", 3, space="PSUM")
    ffhT = []
    for m in range(FFT):
        wt = w1p.tile([P, DT_N, P], MDT, name="w1c", tag="w1c", bufs=4)
        (nc.sync if m % 2 == 0 else nc.scalar).dma_start(
            out=wt, in_=w1[:, m * P:(m + 1) * P].rearrange("(ko p) f -> p ko f", p=P).bitcast(MDT))
        ps = fps.tile([P, 512], F32, name="fps", tag="fps", bufs=3)
        for kk in range(DT_N):
            _mm(nc, ps, wt[:, kk, :], y2nT[kk],
                start=(kk == 0), stop=(kk == DT_N - 1))
        ft = ffhp.tile([P, ROWS], MDT, name="ffhT", tag="ffhT", bufs=FFT)
        nc.vector.tensor_scalar(out=ft, in0=ps,
                                scalar1=b1_t[:, m:m + 1], scalar2=0.0,
                                op0=mybir.AluOpType.add,
                                op1=mybir.AluOpType.max)
        ffhT.append(ft)
    cm_fps.__exit__(None, None, None)
    cm_w1.__exit__(None, None, None)
    cm_y2T.__exit__(None, None, None)

    # ---------------- stage H: FFN2 + residual (SBUF accumulation) ----------
    cm_w2, w2p = open_pool("w2p", 10, side="right")
    cm_cps, cps = open_pool("cps", 3, space="PSUM")
    for tm in range(QT_N):
        nc.vector.tensor_add(out=y_acc[tm], in0=y1[tm], in1=b2_t)
    KC = 8
    for chunk in range(FFT // KC):
        w2_t = []
        for kk in range(KC):
            m = chunk * KC + kk
            t = w2p.tile([P, D], MDT, name="w2_t", tag="w2_t", bufs=10)
            (nc.sync if m % 2 == 0 else nc.scalar).dma_start(
                out=t, in_=w2[m * P:(m + 1) * P, :].bitcast(MDT))
            w2_t.append(t)
        for tm in range(QT_N):
            for n in range(2):
                ps = cps.tile([P, 384], F32, name="cps", tag="cps", bufs=3)
                for kk in range(KC):
                    m = chunk * KC + kk
                    _mm(nc, ps, ffhT[m][:, tm * P:(tm + 1) * P],
                        w2_t[kk][:, n * 384:(n + 1) * 384],
                        start=(kk == 0), stop=(kk == KC - 1))
                nc.vector.tensor_add(out=y_acc[tm][:, n * 384:(n + 1) * 384],
                                     in0=y_acc[tm][:, n * 384:(n + 1) * 384],
                                     in1=ps)
    for tm in range(QT_N):
        nc.sync.dma_start(out=y[tm * P:(tm + 1) * P, :], in_=y_acc[tm])
    cm_cps.__exit__(None, None, None)
    cm_w2.__exit__(None, None, None)
    cm_ffh.__exit__(None, None, None)
    cm_y1.__exit__(None, None, None)
    cm_small.__exit__(None, None, None)
    cm_const.__exit__(None, None, None)


# ---------------------------------------------------------------------------
# host side
# ---------------------------------------------------------------------------
_NC_CACHE = {}


def _bias_flags(in_maps):
    m = in_maps[0]
    return frozenset(f for f in "qkvo" if not np.any(m["b" + f]))


def _get_nc(skip_bias=frozenset()):
    key = (MM_DT, skip_bias)
    if key not in _NC_CACHE:
        _NC_CACHE[key] = build_nc(skip_bias=skip_bias)
    return _NC_CACHE[key]


def _make_in_maps(inputs):
    return _prep_inputs(**{k: np.asarray(v) for k, v in inputs.items()})


def _prep_inputs(x, attn_mask, Wq, Wk, Wv, Wo, bo, ln1_g, ln1_b, ln2_g, ln2_b,
                 W1, b1, W2, b2):
    x = np.asarray(x, dtype=np.float32)
    attn_mask = np.asarray(attn_mask)
    f64 = np.float64
    g1 = np.asarray(ln1_g, f64)
    lb1 = np.asarray(ln1_b, f64)
    g2 = np.asarray(ln2_g, f64)
    lb2 = np.asarray(ln2_b, f64)
    Wq = np.asarray(Wq, f64)
    Wk = np.asarray(Wk, f64)
    Wv = np.asarray(Wv, f64)
    s = float(D) ** -0.5

    import ml_dtypes
    wq_e = ((g1[:, None] * Wq) * s).astype(ml_dtypes.bfloat16)
    bq_e = ((lb1 @ Wq) * s).astype(np.float32)[None, :]
    wk_e = (g1[:, None] * Wk).astype(ml_dtypes.bfloat16)
    bk_e = (lb1 @ Wk).astype(np.float32)[None, :]
    wv_e = (g1[:, None] * Wv).astype(ml_dtypes.bfloat16)
    bv_e = (lb1 @ Wv).astype(np.float32)[None, :]
    wo_e = np.asarray(Wo, np.float32)
    bo_e = np.asarray(bo, np.float32)[None, :]
    W1_64 = np.asarray(W1, f64)
    w1_e = (g2[:, None] * W1_64).astype(np.float32)
    b1_e = (np.asarray(b1, f64) + lb2 @ W1_64).astype(np.float32)
    w2_e = np.asarray(W2, np.float32)
    b2_e = np.asarray(b2, np.float32)[None, :]

    maskf = attn_mask.astype(np.float32)

    in_maps = []
    for c in range(NCORES):
        b = c // CPB
        r0 = (c % CPB) * ROWS
        in_maps.append({
            "xb": np.ascontiguousarray(np.roll(x[b], -r0, axis=0)),
            "xsl": np.ascontiguousarray(x[b, r0:r0 + ROWS]),
            "maskf": np.ascontiguousarray(np.roll(maskf[b], -r0)[:, None]),
            "wq": wq_e, "wk": wk_e, "wv": wv_e, "wo": wo_e,
            "w1": w1_e, "w2": w2_e,
            "bq": bq_e, "bk": bk_e, "bv": bv_e, "bo": bo_e,
            "b1": b1_e, "b2": b2_e,
        })
    return in_maps


def kernel(**inputs):
    in_maps = _make_in_maps(inputs)
    nc = _get_nc(_bias_flags(in_maps))
    res = run_bass_kernel_spmd(nc, in_maps, list(range(NCORES)))

    out = np.empty((B, T, D), dtype=np.float32)
    for c in range(NCORES):
        b = c // CPB
        r0 = (c % CPB) * ROWS
        out[b, r0:r0 + ROWS] = res.results[c]["y"]
    return out



# revision 30
# speedup vs baseline: 202.5522x; 1.0282x over previous
"""Trainium2 Bass kernel for a dense transformer block (B=2, T=2048, D=768, H=12).

Sharding: 8 cores, each owns 512 contiguous tokens of one batch element
(4 cores per batch).  Each core receives its batch's full token stream
(rotated so its own 512 query rows come first), computes K/V for all 2048
keys of that batch locally (no cross-core communication), and attention +
FFN for its own 512 rows.  Host gathers the 8 row-slices.

LayerNorm affine params and the attention scale are folded into the weight
matrices on the host, so the device only computes pure (x-mean)*rstd
normalizations.
"""

import os
import numpy as np

import concourse.bass as bass
import concourse.tile as tile
from concourse import bacc, mybir
from concourse.bass_utils import run_bass_kernel_spmd
from concourse.masks import make_identity

F32 = mybir.dt.float32
BF = mybir.dt.bfloat16

D = 768
H = 12
HS = 64
B = 2
T = 2048
P = 128
NCORES = 8
CPB = NCORES // B          # cores per batch
ROWS = T // CPB            # 512 query rows per core
ST = T // P                # 16 key tiles
QT_N = ROWS // P           # 4 query tiles
DT_N = D // P              # 6 feature tiles
FF = 4 * D                 # 3072
FFT = FF // P              # 24 ff tiles
HP = H // 2                # 6 head pairs
EPS = 1e-5

# matmul input dtype: float32 (exact, 4 cyc/row), float32r (~1 cyc/row),
# or bfloat16 (1 cyc/row, half the SBUF/DMA traffic)
MM_DT = {
    "f32": mybir.dt.float32,
    "f32r": mybir.dt.float32r,
    "bf16": mybir.dt.bfloat16,
}[os.environ.get("KERNEL_MM_DT", "bf16")]


MDT = MM_DT  # dtype of tiles that feed matmuls
RDT = mybir.dt.float32r  # softmax-denominator path keeps f32 bits


def _mm(nc, out, lhsT, rhs, **kw):
    nc.tensor.matmul(out, lhsT, rhs, **kw)


def _copy(eng, out, in_):
    if hasattr(eng, "tensor_copy"):
        eng.tensor_copy(out=out, in_=in_)
    else:
        eng.copy(out=out, in_=in_)


def build_nc(reps=None, skip_bias=(), loop=False):
    nc = bacc.Bacc("TRN2", target_bir_lowering=False, debug=False, num_devices=NCORES)

    xb = nc.declare_dram_parameter("xb", [T, D], BF, isOutput=False)
    xsl = nc.declare_dram_parameter("xsl", [ROWS, D], F32, isOutput=False)
    maskf = nc.declare_dram_parameter("maskf", [T, 1], F32, isOutput=False)
    wq = nc.declare_dram_parameter("wq", [D, D], BF, isOutput=False)
    wk = nc.declare_dram_parameter("wk", [D, D], BF, isOutput=False)
    wv = nc.declare_dram_parameter("wv", [D, D], BF, isOutput=False)
    wo = nc.declare_dram_parameter("wo", [D, D], BF, isOutput=False)
    w1 = nc.declare_dram_parameter("w1", [D, FF], BF, isOutput=False)
    w2 = nc.declare_dram_parameter("w2", [FF, D], BF, isOutput=False)
    bq = nc.declare_dram_parameter("bq", [1, D], BF, isOutput=False)
    bk = nc.declare_dram_parameter("bk", [1, D], BF, isOutput=False)
    bv = nc.declare_dram_parameter("bv", [1, D], BF, isOutput=False)
    bo = nc.declare_dram_parameter("bo", [1, D], BF, isOutput=False)
    b1 = nc.declare_dram_parameter("b1", [FF], F32, isOutput=False)
    b2 = nc.declare_dram_parameter("b2", [1, D], F32, isOutput=False)
    y = nc.declare_dram_parameter("y", [ROWS, D], F32, isOutput=True)

    if reps is None:
        reps = int(os.environ.get("KERNEL_REPS", "1"))
    with tile.TileContext(nc) as tc, \
            nc.allow_low_precision(reason="f32r-rounded matmul operands"):
        if loop:
            # hardware loop: body emitted once, executed `reps` times
            with tc.For_i(0, reps):
                _emit(nc, tc, xb, xsl, maskf, wq, wk, wv, wo, w1, w2,
                      bq, bk, bv, bo, b1, b2, y, skip_bias=frozenset(skip_bias))
        else:
            for r in range(reps):
                if r:
                    tc.strict_bb_all_engine_barrier()
                _emit(nc, tc, xb, xsl, maskf, wq, wk, wv, wo, w1, w2,
                      bq, bk, bv, bo, b1, b2, y, skip_bias=frozenset(skip_bias))
    nc.compile()
    return nc


def _rstd_newton(nc, pool, ve, out_y, three_t, c15_t, nv):
    """out_y[:, i] = 1/sqrt(ve[:, i]) via 2 Newton steps from y0 = 2/(1+ve),
    all on DVE native ops (no ACT table traffic; Sqrt would evict the Exp
    table mid-attention).  y0 guarantees convergence for any ve > 0; two
    steps give ~1e-5 relative accuracy for the ve ~ 1 seen in LayerNorm.
    Batched over nv tiles' variances — one chain serves a whole block."""
    z = pool.tile([P, nv], F32, name="lnz", tag="lnz", bufs=2)
    w = pool.tile([P, nv], F32, name="lnw", tag="lnw", bufs=2)
    nc.vector.tensor_scalar_add(out=z, in0=ve, scalar1=1.0)
    nc.vector.reciprocal(out=z, in_=z)
    # y1 = z * (3 - 4 * ve * z^2)
    nc.vector.tensor_mul(out=w, in0=z, in1=z)
    nc.vector.tensor_mul(out=w, in0=w, in1=ve)
    nc.vector.scalar_tensor_tensor(out=w, in0=w, scalar=-4.0,
                                   in1=three_t.to_broadcast((P, nv)),
                                   op0=mybir.AluOpType.mult,
                                   op1=mybir.AluOpType.add)
    nc.vector.tensor_mul(out=z, in0=z, in1=w)
    # y2 = y1 * (1.5 - 0.5 * ve * y1^2)
    nc.vector.tensor_mul(out=w, in0=z, in1=z)
    nc.vector.tensor_mul(out=w, in0=w, in1=ve)
    nc.vector.scalar_tensor_tensor(out=w, in0=w, scalar=-0.5,
                                   in1=c15_t.to_broadcast((P, nv)),
                                   op0=mybir.AluOpType.mult,
                                   op1=mybir.AluOpType.add)
    nc.vector.tensor_mul(out=out_y, in0=z, in1=w)


def _ln_block(nc, pool, xts, outs, lncst, apply_engines, stat_acts):
    """(x-mean)*rstd for a block of [P, D] token-major tiles.  Per-tile
    stats run on DVE (bn_stats) or ACT (Square/Identity accumulation,
    resident in every activation table so the Exp table never reloads);
    the rstd Newton chain is batched across the whole block on DVE.
    apply_engines[i] does tile i's normalize; ACT applies use bias/scale."""
    three_t, c15_t = lncst
    nb = len(xts)
    mvb = pool.tile([P, nb, 2], F32, name="lnmv", tag="lnmv", bufs=2)
    rstd = pool.tile([P, nb], F32, name="lnstd", tag="lnstd", bufs=2)
    for i, xt in enumerate(xts):
        if stat_acts[i]:
            sums = pool.tile([P, 2], F32, name="lnsums", tag="lnsums", bufs=4)
            nc.scalar.activation(out=outs[i], in_=xt,
                                 func=mybir.ActivationFunctionType.Square,
                                 accum_out=sums[:, 1:2])
            nc.scalar.activation(out=outs[i], in_=xt,
                                 func=mybir.ActivationFunctionType.Identity,
                                 accum_out=sums[:, 0:1])
            # mean = sum/D ; var = sumsq/D - mean^2
            m2 = pool.tile([P, 1], F32, name="lnm2", tag="lnm2", bufs=4)
            nc.vector.tensor_scalar_mul(out=mvb[:, i, :], in0=sums,
                                        scalar1=1.0 / D)
            nc.vector.tensor_mul(out=m2, in0=mvb[:, i, 0:1],
                                 in1=mvb[:, i, 0:1])
            nc.vector.tensor_sub(out=mvb[:, i, 1:2], in0=mvb[:, i, 1:2],
                                 in1=m2)
        else:
            stats = pool.tile([P, 3, 6], F32, name="lnstats", tag="lnstats",
                              bufs=4)
            for g in range(3):
                nc.vector.bn_stats(out=stats[:, g, :],
                                   in_=xt[:, g * 256:(g + 1) * 256])
            nc.vector.bn_aggr(out=mvb[:, i, :], in_=stats)
    nc.vector.tensor_scalar_add(out=mvb[:, :, 1], in0=mvb[:, :, 1],
                                scalar1=EPS)
    _rstd_newton(nc, pool, mvb[:, :, 1], rstd, three_t, c15_t, nb)
    for i, xt in enumerate(xts):
        eng = apply_engines[i]
        if eng is nc.scalar:
            nmr = pool.tile([P, 1], F32, name="lnnmr", tag="lnnmr", bufs=4)
            nc.vector.scalar_tensor_tensor(out=nmr, in0=mvb[:, i, 0:1],
                                           scalar=-1.0, in1=rstd[:, i:i + 1],
                                           op0=mybir.AluOpType.mult,
                                           op1=mybir.AluOpType.mult)
            nc.scalar.activation(out=outs[i], in_=xt,
                                 func=mybir.ActivationFunctionType.Identity,
                                 bias=nmr, scale=rstd[:, i:i + 1])
        else:
            eng.tensor_scalar(out=outs[i], in0=xt,
                              scalar1=mvb[:, i, 0:1],
                              scalar2=rstd[:, i:i + 1],
                              op0=mybir.AluOpType.subtract,
                              op1=mybir.AluOpType.mult)


def _emit(nc, tc, xb, xsl, maskf, wq, wk, wv, wo, w1, w2,
          bq, bk, bv, bo, b1, b2, y, skip_bias=frozenset()):
    # Pools are released in strict LIFO order per (space, side) stack; the
    # open/close sequence below alternates sides so that overlapping
    # non-nested lifetimes land on different stacks.
    def open_pool(name, bufs, space="SBUF", side=None):
        cm = tc.tile_pool(name=name, bufs=bufs, space=space, side=side)
        return cm, cm.__enter__()

    # ---------------- constants ----------------
    cm_const, consts = open_pool("consts", 1)
    identity_r = consts.tile([P, P], MDT, name="identity_r", tag="identity_r")
    make_identity(nc, identity_r)
    ones_st = consts.tile([1, 512 + 3 * P], F32, name="ones_st", tag="ones_st")
    nc.vector.memset(ones_st[:, 0:512 + P], 1.0)
    nc.vector.memset(ones_st[:, 512 + P:], 0.0)
    nc.vector.memset(ones_st[:, 512 + P:512 + P + 64], 1.0)
    nc.vector.memset(ones_st[:, 512 + 2 * P + 64:], 1.0)
    ones_row = consts.tile([1, 512], MDT, name="ones_row", tag="ones_row")
    nc.vector.tensor_copy(out=ones_row, in_=ones_st[:, 0:512])
    onesfull = consts.tile([1, P], MDT, name="onesfull", tag="onesfull")
    nc.vector.tensor_copy(out=onesfull, in_=ones_st[:, 512:512 + P])
    onesA = consts.tile([1, P], RDT, name="onesA", tag="onesA")
    nc.vector.tensor_copy(out=onesA, in_=ones_st[:, 512 + P:512 + 2 * P])
    onesB = consts.tile([1, P], RDT, name="onesB", tag="onesB")
    nc.vector.tensor_copy(out=onesB, in_=ones_st[:, 512 + 2 * P:512 + 3 * P])
    three_t = consts.tile([P, 1], F32, name="three_t", tag="three_t")
    nc.vector.memset(three_t, 3.0)
    c15_t = consts.tile([P, 1], F32, name="c15_t", tag="c15_t")
    nc.vector.memset(c15_t, 1.5)
    lncst = (three_t, c15_t)
    ones_col = consts.tile([P, H], F32, name="ones_col", tag="ones_col")
    nc.vector.memset(ones_col, 1.0)

    cm_small, small = open_pool("small", 4)

    # ---------------- pool layout -----------------------------------------
    # Left stack  : consts, small, KT, QT, V, expp, dnp, xn, xraw | y1, ffh
    # Right stack : aU, wop, xnT, wkp | w1p | w2p | y2T
    #   (xnT/wkp close after all K/Q/V projections — front-loaded into
    #    pairs 0-1 — freeing their space for the full W1/W2 prefetch)
    # PSUM        : mmps, scps, tps (swapped for avps after pair 0) | pps,
    #               tps2, fps, cps
    cm_aU, aUp = open_pool("aU", DT_N, side="right")
    attnUT = [aUp.tile([P, ROWS], MDT, name="attnUT", tag="attnUT", bufs=DT_N)
              for _ in range(DT_N)]
    cm_wo, wop = open_pool("wop", DT_N, side="right")
    cm_xnT, xnTp = open_pool("xnT", DT_N, side="right")
    xnT = [xnTp.tile([P, T], BF, name="xnT", tag="xnT", bufs=DT_N)
           for _ in range(DT_N)]
    cm_KT, KTp = open_pool("KT", DT_N)
    KT = [KTp.tile([P, T], BF, name="KT", tag="KT", bufs=DT_N)
          for _ in range(DT_N)]
    cm_QT, QTp = open_pool("QT", DT_N)
    QT = [QTp.tile([P, ROWS], BF, name="QT", tag="QT", bufs=DT_N)
          for _ in range(DT_N)]
    cm_V, Vp = open_pool("V", ST)
    V_ext = [Vp.tile([P, H, HS + 1], BF, name="V_ext", tag="V_ext", bufs=ST)
             for _ in range(ST)]
    EXPB = 16
    cm_exp, expp = open_pool("expp", EXPB)
    cm_dn, dnp = open_pool("dnp", 3)
    cm_xn, xnp = open_pool("xn", 6)
    cm_xraw, xrawp = open_pool("xraw", ST)
    cm_wk, wkp = open_pool("wkp", 3 * DT_N, side="right")

    # ---------------- DMA schedule -----------------------------------------
    # Two HWDGE queues (sync/scalar) carry, in order: the core's own four x
    # tiles, Wk, Wq, two more x tiles, Wv, then the remaining x tiles.  The
    # back-half weights (Wo, W1, W2) are enqueued when attention pair 2
    # starts.  Small consts ride the gpsimd SWDGE queue.
    xraw = []

    def load_x(i):
        t = xrawp.tile([P, D], BF, name="xraw", tag="xraw", bufs=ST)
        nc.sync.dma_start(out=t, in_=xb[i * P:(i + 1) * P, :])
        xraw.append(t)

    def load_w6(w, pool, tag, bufs):
        out = []
        for kk in range(DT_N):
            t = pool.tile([P, D], BF, name=tag, tag=tag, bufs=bufs)
            nc.gpsimd.dma_start(out=t, in_=w[kk * P:(kk + 1) * P, :])
            out.append(t)
        return out

    for i in range(ST):
        load_x(i)
    wk_t = load_w6(wk, wkp, "wk_t", 3 * DT_N)
    wq_t = load_w6(wq, wkp, "wk_t", 3 * DT_N)
    wv_t = load_w6(wv, wkp, "wk_t", 3 * DT_N)

    mask_all = consts.tile([P, ST], F32, name="mask_all", tag="mask_all")
    nc.gpsimd.dma_start(out=mask_all, in_=maskf[:, :].rearrange("(n p) o -> p (n o)", p=P))
    bq_t = bk_t = bv_t = bo_t = None
    for f, src_p in (("q", bq), ("k", bk), ("v", bv), ("o", bo)):
        if f in skip_bias:
            continue
        t = consts.tile([1, D], MDT, name=f"b{f}_t", tag=f"b{f}_t")
        nc.gpsimd.dma_start(out=t, in_=src_p[:, :])
        if f == "q":
            bq_t = t
        elif f == "k":
            bk_t = t
        elif f == "v":
            bv_t = t
        else:
            bo_t = t
    b1_t = consts.tile([P, FFT], F32, name="b1_t", tag="b1_t")
    nc.gpsimd.dma_start(out=b1_t, in_=b1[:].rearrange("(a p) -> p a", p=P))
    b2_t = consts.tile([P, D], F32, name="b2_t", tag="b2_t")
    nc.gpsimd.dma_start(out=b2_t, in_=b2[:, :].to_broadcast((P, D)))
    wo_t = load_w6(wo, wop, "wo_t", DT_N)

    # ------------- attention emit helpers -----------------------------------
    cm_mmps, mmps = open_pool("mmps", 2, space="PSUM")
    cm_tps, tps = open_pool("tps", 6, space="PSUM")
    cpy = (nc.vector, nc.scalar)

    def ln_block(i0):
        tiles = [xraw[i0 + j] for j in range(4)]
        outs = [xnp.tile([P, D], MDT, name="xn", tag="xn", bufs=6)
                for _ in range(4)]
        _ln_block(nc, small, tiles, outs, lncst,
                  apply_engines=(nc.scalar, nc.scalar, nc.scalar, nc.scalar),
                  stat_acts=(False, False, False, False))
        for j in range(4):
            i = i0 + j
            for k in range(DT_N):
                pt = tps.tile([P, P], MDT, name="tp", tag="tp", bufs=6)
                nc.tensor.transpose(pt, outs[j][:, k * P:(k + 1) * P],
                                    identity_r)
                _copy(cpy[(i * DT_N + k) % 2],
                      xnT[k][:, i * P:(i + 1) * P], pt)

    def emit_kt(k, n):
        ps = mmps.tile([P, 512], F32, name="mmps", tag="mmps", bufs=2)
        for kk in range(DT_N):
            _mm(nc, ps, wk_t[kk][:, k * P:(k + 1) * P],
                xnT[kk][:, n * 512:(n + 1) * 512], start=(kk == 0),
                stop=(kk == DT_N - 1 and "k" in skip_bias))
        if "k" not in skip_bias:
            _mm(nc, ps, bk_t[:, k * P:(k + 1) * P], ones_row,
                start=False, stop=True)
        _copy(cpy[(k + n) % 2], KT[k][:, n * 512:(n + 1) * 512], ps)

    def emit_qt(k):
        ps = mmps.tile([P, 512], F32, name="mmps", tag="mmps", bufs=2)
        for kk in range(DT_N):
            _mm(nc, ps, wq_t[kk][:, k * P:(k + 1) * P],
                xnT[kk][:, 0:ROWS], start=(kk == 0),
                stop=(kk == DT_N - 1 and "q" in skip_bias))
        if "q" not in skip_bias:
            _mm(nc, ps, bq_t[:, k * P:(k + 1) * P], ones_row[:, 0:ROWS],
                start=False, stop=True)
        _copy(cpy[k % 2], QT[k], ps)

    def emit_v(i):
        for half in range(2):
            ps = mmps.tile([P, 384], F32, name="mmps", tag="mmps", bufs=2)
            for kk in range(DT_N):
                _mm(nc, ps, xnT[kk][:, i * P:(i + 1) * P],
                    wv_t[kk][:, half * 384:(half + 1) * 384],
                    start=(kk == 0),
                    stop=(kk == DT_N - 1 and "v" in skip_bias))
            if "v" not in skip_bias:
                _mm(nc, ps, onesfull, bv_t[:, half * 384:(half + 1) * 384],
                    start=False, stop=True)
            if (i + half) % 2 == 0:
                nc.vector.tensor_scalar_mul(
                    out=V_ext[i][:, half * 6:(half + 1) * 6, 0:HS],
                    in0=ps.rearrange("p (h v) -> p h v", h=6),
                    scalar1=mask_all[:, i:i + 1])
            else:
                nc.scalar.activation(
                    out=V_ext[i][:, half * 6:(half + 1) * 6, 0:HS],
                    in_=ps.rearrange("p (h v) -> p h v", h=6),
                    func=mybir.ActivationFunctionType.Copy,
                    scale=mask_all[:, i:i + 1])
        nc.gpsimd.tensor_scalar_mul(out=V_ext[i][:, :, HS:HS + 1],
                                    in0=ones_col[:, 0:H].unsqueeze(2),
                                    scalar1=mask_all[:, i:i + 1])

    ets = {}
    avs = {}

    def emit_scores_step(kp, jj, et):
        ps0 = scps.tile([P, 2, 512], F32, name="scps", tag="scps", bufs=2)
        ps1 = scps.tile([P, 2, 512], F32, name="scps", tag="scps", bufs=2)
        for u in range(2):
            j = jj * 2 + u
            _mm(nc, ps0[:, u, :], KT[kp][0:64, j * P:(j + 1) * P],
                QT[kp][0:64, :], start=True, stop=True)
            _mm(nc, ps1[:, u, :], KT[kp][64:128, j * P:(j + 1) * P],
                QT[kp][64:128, :], start=True, stop=True)
        e0 = expp.tile([P, 2, 512], BF, name="expT", tag="expT", bufs=EXPB)
        e1 = expp.tile([P, 2, 512], BF, name="expT", tag="expT", bufs=EXPB)
        nc.scalar.activation(out=e0, in_=ps0,
                             func=mybir.ActivationFunctionType.Exp)
        nc.scalar.activation(out=e1, in_=ps1,
                             func=mybir.ActivationFunctionType.Exp)
        et.append((e0, e1))

    def emit_attnv_step(kp, jj):
        av0, av1 = avs[kp]
        et = ets[kp]
        for u in range(2):
            j = jj * 2 + u
            _mm(nc, av0, V_ext[j][:, 2 * kp, :], et[jj][0][:, u, :],
                start=(j == 0), stop=(j == ST - 1))
            _mm(nc, av1, V_ext[j][:, 2 * kp + 1, :], et[jj][1][:, u, :],
                start=(j == 0), stop=(j == ST - 1))

    def finish_pair(kp):
        d_pair = []
        for half, av in ((0, avs[kp][0]), (1, avs[kp][1])):
            nc.vector.tensor_copy(out=attnUT[kp][half * 64:half * 64 + 64, :],
                                  in_=av[0:64, :])
            d_sb = dnp.tile([1, 512], RDT, name="d_sb", tag="d_sb", bufs=3)
            nc.vector.tensor_copy(out=d_sb, in_=av[64:65, :])
            d_pair.append(d_sb)
        # R broadcast reuses a scores psum slot; reciprocal on 128 lanes
        rp_t = scps.tile([P, 2, 512], F32, name="scps", tag="scps", bufs=2)
        rp = rp_t[:, 0, :]
        _mm(nc, rp, onesA, d_pair[0], start=True, stop=False)
        _mm(nc, rp, onesB, d_pair[1], start=False, stop=True)
        rr = dnp.tile([P, 512], F32, name="rrec", tag="rrec", bufs=2)
        nc.vector.reciprocal(out=rr, in_=rp)
        nc.vector.tensor_mul(out=attnUT[kp], in0=attnUT[kp], in1=rr)

    # ------------- stages B+C+D: LN/transpose + QKV + attention -------------
    # Stage B normalizes+transposes each x tile as it lands and interleaves
    # the K/Q/V projections whose inputs are already resident, so the PE
    # follows the DMA stream.  Scores start once KT[0]/QT[0] exist;
    # KT/QT[kp+1] are produced during pair kp; attnV for pair kp-1 is
    # emitted *before* the scores of the current step so the 18-deep exp
    # pool rotates exactly one pair behind.
    for blk in range(QT_N):
        ln_block(4 * blk)
        emit_kt(0, blk)
        if blk == 0:
            emit_qt(0)
        elif blk == 1:
            emit_qt(1)
        else:
            emit_v(2 * (blk - 2))
            emit_v(2 * (blk - 2) + 1)
    cm_tps.__exit__(None, None, None)
    cm_xraw.__exit__(None, None, None)
    cm_xn.__exit__(None, None, None)
    cm_scps, scps = open_pool("scps", 2, space="PSUM")
    cm_avps, avps = open_pool("avps", 2, space="PSUM")
    v_next = 4

    def emit_vs(n):
        nonlocal v_next
        for _ in range(n):
            if v_next < ST:
                emit_v(v_next)
                v_next += 1

    for kp in range(HP + 1):
        if kp < HP:
            ets[kp] = []
        if kp >= 1:
            avs[kp - 1] = (
                avps.tile([HS + 1, 512], F32, name="avps", tag="avps", bufs=2),
                avps.tile([HS + 1, 512], F32, name="avps", tag="avps", bufs=2))
        for jj in range(ST // 2):
            if kp >= 1:
                emit_attnv_step(kp - 1, jj)
            if kp < HP:
                emit_scores_step(kp, jj, ets[kp])
            if kp == 0:
                if jj >= 2:
                    emit_vs(2)
                if jj < 4:
                    emit_kt(1, jj)
            elif kp + 1 < HP:
                if jj < 4:
                    emit_kt(kp + 1, jj)
                elif jj == 4:
                    emit_qt(kp + 1)
        if kp >= 1:
            finish_pair(kp - 1)
            del ets[kp - 1]
    cm_avps.__exit__(None, None, None)
    cm_scps.__exit__(None, None, None)
    cm_mmps.__exit__(None, None, None)
    cm_dn.__exit__(None, None, None)
    cm_exp.__exit__(None, None, None)
    cm_V.__exit__(None, None, None)
    cm_QT.__exit__(None, None, None)
    cm_KT.__exit__(None, None, None)

    # ---------------- stage E: out-projection + residual --------------------
    cm_y1, y1p = open_pool("y1", 2 * QT_N)
    y1 = [y1p.tile([P, D], F32, name="y1", tag="y1y", bufs=2 * QT_N)
          for _ in range(QT_N)]
    y_acc = [y1p.tile([P, D], F32, name="yacc", tag="y1y", bufs=2 * QT_N)
             for _ in range(QT_N)]
    cm_xsl, xslp = open_pool("xslp", QT_N)
    xsl_t = []
    for i in range(QT_N):
        t = xslp.tile([P, D], F32, name="xsl", tag="xsl", bufs=QT_N)
        nc.sync.dma_start(out=t, in_=xsl[i * P:(i + 1) * P, :])
        xsl_t.append(t)
    cm_pps, pps = open_pool("pps", 4, space="PSUM")
    for tm in range(QT_N):
        for n in range(2):
            ps = pps.tile([P, 384], F32, name="pps", tag="pps", bufs=4)
            for kk in range(DT_N):
                _mm(nc, ps, attnUT[kk][:, tm * P:(tm + 1) * P],
                    wo_t[kk][:, n * 384:(n + 1) * 384],
                    start=(kk == 0),
                    stop=(kk == DT_N - 1 and "o" in skip_bias))
            if "o" not in skip_bias:
                _mm(nc, ps, onesfull, bo_t[:, n * 384:(n + 1) * 384],
                    start=False, stop=True)
            nc.vector.tensor_add(
                out=y1[tm][:, n * 384:(n + 1) * 384],
                in0=ps, in1=xsl_t[tm][:, n * 384:(n + 1) * 384])
    cm_pps.__exit__(None, None, None)

    # ---------------- stage F: LN2 + transpose ------------------------------
    cm_y2T, y2Tp = open_pool("y2T", DT_N, side="right")
    y2nT = [y2Tp.tile([P, ROWS], MDT, name="y2nT", tag="y2nT", bufs=DT_N)
            for _ in range(DT_N)]
    cm_y2, y2p = open_pool("y2", QT_N)
    cm_tps2, tps2 = open_pool("tps2", 6, space="PSUM")
    y2n = [y2p.tile([P, D], MDT, name="y2n", tag="y2n", bufs=QT_N)
           for _ in range(QT_N)]
    _ln_block(nc, small, y1, y2n, lncst,
              apply_engines=(nc.scalar, nc.scalar, nc.scalar, nc.scalar),
              stat_acts=(False, False, False, False))
    for tm in range(QT_N):
        for k in range(DT_N):
            pt = tps2.tile([P, P], MDT, name="tp2", tag="tp2", bufs=6)
            nc.tensor.transpose(pt, y2n[tm][:, k * P:(k + 1) * P],
                                identity_r)
            _copy(cpy[(tm * DT_N + k) % 2],
                  y2nT[k][:, tm * P:(tm + 1) * P], pt)
    cm_tps2.__exit__(None, None, None)
    cm_y2.__exit__(None, None, None)

    # ---------------- stage G: FFN1 -> ffhT (feature-major, bias+relu) ------
    cm_ffh, ffhp = open_pool("ffh", FFT)
    cm_w1p, w1pp = open_pool("w1p", 6, side="right")
    cm_fps, fps = open_pool("fps", 3, space="PSUM")
    ffhT = []
    for m in range(FFT):
        wt = w1pp.tile([P, DT_N, P], MDT, name="w1c", tag="w1c", bufs=6)
        (nc.sync if m % 2 == 0 else nc.gpsimd).dma_start(
            out=wt, in_=w1[:, m * P:(m + 1) * P].rearrange("(ko p) f -> p ko f", p=P))
        ps = fps.tile([P, 512], F32, name="fps", tag="fps", bufs=3)
        for kk in range(DT_N):
            _mm(nc, ps, wt[:, kk, :], y2nT[kk],
                start=(kk == 0), stop=(kk == DT_N - 1))
        ft = ffhp.tile([P, ROWS], MDT, name="ffhT", tag="ffhT", bufs=FFT)
        if m % 2 == 0:
            nc.scalar.activation(out=ft, in_=ps,
                                 func=mybir.ActivationFunctionType.Relu,
                                 bias=b1_t[:, m:m + 1])
        else:
            nc.vector.tensor_scalar(
                out=ft, in0=ps,
                scalar1=b1_t[:, m:m + 1], scalar2=0.0,
                op0=mybir.AluOpType.add,
                op1=mybir.AluOpType.max)
        ffhT.append(ft)
    cm_fps.__exit__(None, None, None)
    cm_w1p.__exit__(None, None, None)
    cm_y2T.__exit__(None, None, None)

    # ---------------- stage H: FFN2 + residual (SBUF accumulation) ----------
    cm_w2p, w2pp = open_pool("w2p", 12, side="right")
    cm_cps, cps = open_pool("cps", 3, space="PSUM")
    for tm in range(QT_N):
        nc.gpsimd.tensor_add(out=y_acc[tm], in0=y1[tm], in1=b2_t)
    KC = 8
    NCH = FFT // KC
    for chunk in range(NCH):
        w2_t = []
        for kk in range(KC):
            m = chunk * KC + kk
            t = w2pp.tile([P, D], MDT, name="w2_t", tag="w2_t", bufs=12)
            (nc.sync if m % 2 == 0 else nc.gpsimd).dma_start(
                out=t, in_=w2[m * P:(m + 1) * P, :])
            w2_t.append(t)
        for tm in range(QT_N):
            for n in range(2):
                ps = cps.tile([P, 384], F32, name="cps", tag="cps", bufs=3)
                for kk in range(KC):
                    m = chunk * KC + kk
                    _mm(nc, ps, ffhT[m][:, tm * P:(tm + 1) * P],
                        w2_t[kk][:, n * 384:(n + 1) * 384],
                        start=(kk == 0), stop=(kk == KC - 1))
                nc.vector.tensor_add(
                    out=y_acc[tm][:, n * 384:(n + 1) * 384],
                    in0=y_acc[tm][:, n * 384:(n + 1) * 384],
                    in1=ps)
            if chunk == NCH - 1:
                (nc.sync if tm % 2 == 0 else nc.scalar).dma_start(
                    out=y[tm * P:(tm + 1) * P, :], in_=y_acc[tm])
    cm_cps.__exit__(None, None, None)
    cm_w2p.__exit__(None, None, None)
    cm_wk.__exit__(None, None, None)
    cm_xnT.__exit__(None, None, None)
    cm_wo.__exit__(None, None, None)
    cm_aU.__exit__(None, None, None)
    cm_ffh.__exit__(None, None, None)
    cm_xsl.__exit__(None, None, None)
    cm_y1.__exit__(None, None, None)
    cm_small.__exit__(None, None, None)
    cm_const.__exit__(None, None, None)


# ---------------------------------------------------------------------------
# host side
# ---------------------------------------------------------------------------
_NC_CACHE = {}


def _bias_flags(in_maps):
    m = in_maps[0]
    return frozenset(f for f in "qkvo" if not np.any(m["b" + f]))


def _get_nc(skip_bias=frozenset()):
    key = (MM_DT, skip_bias)
    if key not in _NC_CACHE:
        _NC_CACHE[key] = build_nc(skip_bias=skip_bias)
    return _NC_CACHE[key]


def _make_in_maps(inputs):
    return _prep_inputs(**{k: np.asarray(v) for k, v in inputs.items()})


def _prep_inputs(x, attn_mask, Wq, Wk, Wv, Wo, bo, ln1_g, ln1_b, ln2_g, ln2_b,
                 W1, b1, W2, b2):
    x = np.asarray(x, dtype=np.float32)
    attn_mask = np.asarray(attn_mask)
    f64 = np.float64
    g1 = np.asarray(ln1_g, f64)
    lb1 = np.asarray(ln1_b, f64)
    g2 = np.asarray(ln2_g, f64)
    lb2 = np.asarray(ln2_b, f64)
    Wq = np.asarray(Wq, f64)
    Wk = np.asarray(Wk, f64)
    Wv = np.asarray(Wv, f64)
    s = float(D) ** -0.5

    import ml_dtypes
    wq_e = ((g1[:, None] * Wq) * s).astype(ml_dtypes.bfloat16)
    bq_e = ((lb1 @ Wq) * s).astype(ml_dtypes.bfloat16)[None, :]
    wk_e = (g1[:, None] * Wk).astype(ml_dtypes.bfloat16)
    bk_e = (lb1 @ Wk).astype(ml_dtypes.bfloat16)[None, :]
    wv_e = (g1[:, None] * Wv).astype(ml_dtypes.bfloat16)
    bv_e = (lb1 @ Wv).astype(ml_dtypes.bfloat16)[None, :]
    wo_e = np.asarray(Wo).astype(ml_dtypes.bfloat16)
    bo_e = np.asarray(bo).astype(ml_dtypes.bfloat16)[None, :]
    W1_64 = np.asarray(W1, f64)
    w1_e = (g2[:, None] * W1_64).astype(ml_dtypes.bfloat16)
    b1_e = (np.asarray(b1, f64) + lb2 @ W1_64).astype(np.float32)
    w2_e = np.asarray(W2).astype(ml_dtypes.bfloat16)
    b2_e = np.asarray(b2, np.float32)[None, :]

    maskf = attn_mask.astype(np.float32)

    in_maps = []
    for c in range(NCORES):
        b = c // CPB
        r0 = (c % CPB) * ROWS
        in_maps.append({
            "xb": np.ascontiguousarray(np.roll(x[b], -r0, axis=0)).astype(ml_dtypes.bfloat16),
            "xsl": np.ascontiguousarray(x[b, r0:r0 + ROWS]),
            "maskf": np.ascontiguousarray(np.roll(maskf[b], -r0)[:, None]),
            "wq": wq_e, "wk": wk_e, "wv": wv_e, "wo": wo_e,
            "w1": w1_e, "w2": w2_e,
            "bq": bq_e, "bk": bk_e, "bv": bv_e, "bo": bo_e,
            "b1": b1_e, "b2": b2_e,
        })
    return in_maps


def kernel(**inputs):
    in_maps = _make_in_maps(inputs)
    nc = _get_nc(_bias_flags(in_maps))
    res = run_bass_kernel_spmd(nc, in_maps, list(range(NCORES)))

    out = np.empty((B, T, D), dtype=np.float32)
    for c in range(NCORES):
        b = c // CPB
        r0 = (c % CPB) * ROWS
        out[b, r0:r0 + ROWS] = res.results[c]["y"]
    return out

